# revision 6
# baseline (speedup 1.0000x reference)
"""ColumnParallelFusedMoeLinear grouped-GEMM kernel for 8 Trainium2 NeuronCores.

Strategy (expert/token parallel):
  Tokens are sorted by expert; m_sizes gives each expert's contiguous row
  range of x.  The host splits the token range into 8 contiguous
  single-expert chunks (balanced m_sizes -> one expert per core; a skewed
  expert gets split across cores).  Each core computes its chunk's
  y_chunk = x_chunk @ weight[e].T and the host scatters rows back.

  Design notes (vs the fp32r x-stationary baseline, 95.3us):
  * bf16 inputs AND bf16 output (host casts) -> HBM traffic halved
    (~10.5 MB/core).  PE rate is 1 column/cycle for both fp32r and bf16,
    so bf16 costs no compute -- it only cuts DMA.  Rel-err ~2.9e-3 << 2e-2.
  * WEIGHT-stationary matmuls: lhsT = wT tile [128k x 128 d_out], moving
    rhs = xT [128k x m tokens].  The moving dim is tokens, so the token
    count needs NO 128-row padding: PE work is 16nt x 8kc x m_pad cols
    ~= 57us/core vs 61.4us for the padded x-stationary layout.
  * Weights are HOST-PACKED into the exact SBUF layout ([partition, free]
    with kc-major 128-col runs per nt) so every weight DMA is a fully
    contiguous per-partition run (4KB) -- the naive strided gather ran at
    ~200 GB/s and starved the PE for ~8us mid-kernel.
  * Output is written as yT [d_out, m_pad] (psum already holds yT tiles);
    the host transposes back.  Stores stream out per nt-pair during
    compute on the scalar HWDGE ring; loads use the sync ring.  The last
    nt is stored per-512-block so the final store (and its ~2us HBM
    receipt) is small.
  * One 3-bank PSUM tile + ONE cast per nt (instead of 3) keeps the
    instruction/semaphore count down.
  * PE warm-up: dummy matmuls on a memset scratch tile run during the
    initial DMA fill so the HAM clock-gate is at 8/8 when real work lands.
"""

import math

import numpy as np

_N_CORES = 8
_P = 128
_MBLK = 512  # PSUM bank width in fp32

_program_cache = {}


def _build_program(m_pad, d_in, d_out, warm_mms=8):
    import concourse.mybir as mybir
    import concourse.tile as tile
    from concourse import bacc

    kc_n = d_in // _P           # contraction chunks of 128
    nt_n = d_out // _P          # output-feature tiles of 128
    blocks = [(s, min(s + _MBLK, m_pad)) for s in range(0, m_pad, _MBLK)]
    nblk = len(blocks)

    nc = bacc.Bacc("TRN2", target_bir_lowering=False, debug=False)
    xT = nc.dram_tensor("xT", [d_in, m_pad], mybir.dt.bfloat16, kind="ExternalInput")
    # wP: host-packed weights. wP[p, (nt*kc_n + kc)*128 + j] =
    #     weight[e].T[kc*128 + p, nt*128 + j]
    wP = nc.dram_tensor("wP", [_P, nt_n * kc_n * _P], mybir.dt.bfloat16,
                        kind="ExternalInput")
    yT = nc.dram_tensor("yT", [d_out, m_pad], mybir.dt.bfloat16,
                        kind="ExternalOutput")

    xT3 = xT.rearrange("(kc p) m -> kc p m", p=_P)
    yT3 = yT.rearrange("(nt p) m -> nt p m", p=_P)

    # weight DMA groups (by nt): two singles up front so the first matmuls
    # aren't gated on a big transfer, pairs after.
    wgroups = []
    nt = 0
    while nt < nt_n:
        g = 1 if len(wgroups) < 2 else 2
        g = min(g, nt_n - nt)
        wgroups.append((nt, nt + g))
        nt += g
    gi_of_nt = {}
    for gi, (n0, n1) in enumerate(wgroups):
        for t in range(n0, n1):
            gi_of_nt[t] = gi

    with tile.TileContext(nc) as tc:
        with (
            tc.tile_pool(name="xw", bufs=1) as xwpool,
            tc.tile_pool(name="out", bufs=4) as outpool,
            tc.tile_pool(name="psum", bufs=2, space="PSUM") as psumpool,
            tc.tile_pool(name="wps", bufs=1, space="PSUM") as wpspool,
        ):
            # ---- PE warm-up on a zeroed scratch tile (no data deps) ----
            if warm_mms:
                scratch = xwpool.tile([_P, 640], mybir.dt.bfloat16, tag="scratch")
                nc.gpsimd.memset(scratch[:], 0.0)
                wps = wpspool.tile([_P, _MBLK], mybir.dt.float32, tag="wps")
                for _ in range(warm_mms):
                    nc.tensor.matmul(wps[:], scratch[:, 0:_P], scratch[:, _P:640],
                                     start=True, stop=True)

            # ---- input DMAs (sync/SP HWDGE ring), in PE need-order ----
            xsb = xwpool.tile([_P, kc_n * m_pad], mybir.dt.bfloat16, tag="x")
            wsb = {}

            def load_w(gi):
                n0, n1 = wgroups[gi]
                t = xwpool.tile([_P, (n1 - n0) * kc_n * _P], mybir.dt.bfloat16,
                                tag=f"w{gi}")
                nc.sync.dma_start(t[:], wP[:, n0 * kc_n * _P:n1 * kc_n * _P])
                wsb[gi] = t

            nc.sync.dma_start(xsb[:, 0:m_pad], xT3[0])
            load_w(0)
            for kc in range(1, kc_n):
                nc.sync.dma_start(xsb[:, kc * m_pad:(kc + 1) * m_pad], xT3[kc])
            for gi in range(1, len(wgroups)):
                load_w(gi)

            # ---- compute + cast + store ----
            # stores per nt-pair; the final nt per-block so the last store
            # is small.
            for t in range(nt_n):
                gi = gi_of_nt[t]
                n0 = wgroups[gi][0]
                ps = psumpool.tile([_P, nblk * _MBLK], mybir.dt.float32,
                                   tag="ps", name=f"ps_{t}")
                for kc in range(kc_n):
                    off = ((t - n0) * kc_n + kc) * _P
                    lhsT = wsb[gi][:, off:off + _P]
                    for bi, (s, e) in enumerate(blocks):
                        nc.tensor.matmul(
                            ps[:, bi * _MBLK:bi * _MBLK + (e - s)],
                            lhsT,
                            xsb[:, kc * m_pad + s:kc * m_pad + e],
                            start=(kc == 0),
                            stop=(kc == kc_n - 1),
                        )
                last = (t == nt_n - 1)
                # pair stores only when both halves will be produced before
                # the last nt (which stores per-block)
                paired = (t // 2) * 2 + 1 <= nt_n - 2
                if not last:
                    if not paired:
                        o = outpool.tile([_P, m_pad], mybir.dt.bfloat16,
                                         tag="o", name=f"o_{t}")
                        o_half = 0
                    elif t % 2 == 0:
                        o = outpool.tile([_P, 2 * m_pad], mybir.dt.bfloat16,
                                         tag="o", name=f"o_{t}")
                        o_half = 0
                    else:
                        o_half = 1
                    dst = o[:, o_half * m_pad:(o_half + 1) * m_pad]
                    src = ps[:].rearrange("p (b m) -> p b m", m=_MBLK)
                    # single cast over the whole nt (3 banks): AP is
                    # [p, b, cols] with a ragged tail handled by two casts
                    # when m_pad isn't a multiple of 512.
                    if m_pad % _MBLK == 0:
                        nc.vector.tensor_copy(
                            dst.rearrange("p (b m) -> p b m", m=_MBLK), src)
                    else:
                        nb_full = m_pad // _MBLK
                        if nb_full:
                            nc.vector.tensor_copy(
                                dst[:, :nb_full * _MBLK].rearrange(
                                    "p (b m) -> p b m", m=_MBLK),
                                src[:, :nb_full],
                            )
                        s_t, e_t = blocks[-1]
                        nc.vector.tensor_copy(
                            dst[:, s_t:e_t],
                            ps[:, nb_full * _MBLK:nb_full * _MBLK + (e_t - s_t)],
                        )
                    if paired and t % 2 == 1:
                        nc.scalar.dma_start(
                            yT3[t - 1:t + 1].rearrange("t p m -> p t m"),
                            o[:].rearrange("p (t m) -> p t m", t=2),
                        )
                    elif not paired:
                        nc.scalar.dma_start(yT3[t], o[:])
                else:
                    # last nt: per-block cast+store so the tail store is tiny
                    ol = outpool.tile([_P, m_pad], mybir.dt.bfloat16,
                                      tag="ol", name="o_last")
                    for bi, (s, e) in enumerate(blocks):
                        nc.vector.tensor_copy(
                            ol[:, s:e],
                            ps[:, bi * _MBLK:bi * _MBLK + (e - s)])
                        nc.scalar.dma_start(yT3[t][:, s:e], ol[:, s:e])
    nc.compile()
    return nc


# Largest chunk one core handles per SPMD round.
_MAX_CHUNK = 2560


def _plan_chunks(m_sizes, T):
    """Split [0, T) into single-expert chunks, balanced by length.

    Every chunk is <= _MAX_CHUNK rows.  Returns a list of (expert, row0,
    row1) padded with empty (0, 0, 0) chunks to a multiple of _N_CORES,
    or None if there are no rows at all.
    """
    off = np.cumsum(np.asarray(m_sizes, dtype=np.int64))
    starts = np.clip(np.concatenate([[0], off[:-1]]), 0, T)
    ends = np.clip(off, 0, T)
    segs = [(e, int(starts[e]), int(ends[e]))
            for e in range(len(m_sizes)) if ends[e] > starts[e]]
    if not segs:
        return None
    lens = np.array([s1 - s0 for _, s0, s1 in segs], dtype=np.float64)
    n_chunks = np.ceil(lens / _MAX_CHUNK).astype(np.int64)
    total = int(n_chunks.sum())
    spare = (-total) % _N_CORES if total > _N_CORES else _N_CORES - total
    for _ in range(spare):
        i = int(np.argmax(lens / n_chunks))
        n_chunks[i] += 1
    chunks = []
    for (e, s0, s1), k in zip(segs, n_chunks):
        L = s1 - s0
        bounds = [s0 + (L * i) // k for i in range(int(k) + 1)]
        for i in range(int(k)):
            if bounds[i + 1] > bounds[i]:
                chunks.append((e, bounds[i], bounds[i + 1]))
    while len(chunks) % _N_CORES:
        chunks.append((0, 0, 0))
    return chunks


def _pack_weight(wT_e, kc_n, nt_n):
    """wT_e [d_in, d_out] -> [128, nt*kc*128] in the SBUF layout."""
    d_in, d_out = wT_e.shape
    w4 = wT_e.reshape(kc_n, _P, nt_n, _P)          # [kc, p, nt, j]
    return np.ascontiguousarray(
        w4.transpose(1, 2, 0, 3).reshape(_P, nt_n * kc_n * _P))


def kernel(x, weight, m_sizes):
    import ml_dtypes
    from concourse.bass_utils import run_bass_kernel_spmd

    bf16 = ml_dtypes.bfloat16
    x = np.ascontiguousarray(np.asarray(x), dtype=np.float32)
    weight = np.ascontiguousarray(np.asarray(weight), dtype=np.float32)
    m_arr = np.asarray(m_sizes)

    T, d_in = x.shape
    E, d_out, _ = weight.shape

    y = np.zeros((T, d_out), dtype=np.float32)
    chunks = _plan_chunks(m_arr, T)
    if chunks is None:
        return y

    max_len = max(r1 - r0 for _, r0, r1 in chunks)
    m_pad = max(8, int(math.ceil(max_len / 4)) * 4)

    import os
    warm_mms = int(os.environ.get("MOE_WARM_MMS", "8"))
    key = (m_pad, d_in, d_out, warm_mms)
    if key not in _program_cache:
        _program_cache[key] = _build_program(m_pad, d_in, d_out, warm_mms)
    nc = _program_cache[key]

    kc_n = d_in // _P
    nt_n = d_out // _P
    wP_cache = {}
    for round0 in range(0, len(chunks), _N_CORES):
        batch = chunks[round0:round0 + _N_CORES]
        in_maps = []
        for e, r0, r1 in batch:
            xT = np.zeros((d_in, m_pad), dtype=bf16)
            if r1 > r0:
                xT[:, : r1 - r0] = x[r0:r1].T
            if e not in wP_cache:
                wP_cache[e] = _pack_weight(
                    np.ascontiguousarray(weight[e].T).astype(bf16), kc_n, nt_n)
            in_maps.append({"xT": xT, "wP": wP_cache[e]})

        res = run_bass_kernel_spmd(nc, in_maps, core_ids=list(range(_N_CORES)))

        for (e, r0, r1), out in zip(batch, res.results):
            if r1 > r0:
                y[r0:r1] = out["yT"][:, : r1 - r0].T.astype(np.float32)
    return y


# revision 8
# speedup vs baseline: 1.1554x; 1.1554x over previous
"""ColumnParallelFusedMoeLinear grouped-GEMM kernel for 8 Trainium2 NeuronCores.

Strategy (expert/token parallel):
  Tokens are sorted by expert; m_sizes gives each expert's contiguous row
  range of x.  The host splits the token range into 8 contiguous
  single-expert chunks (balanced m_sizes -> one expert per core; a skewed
  expert gets split across cores).  Each core computes its chunk's
  y_chunk = x_chunk @ weight[e].T and the host scatters rows back.

  Design notes (vs the fp32r x-stationary baseline, 95.3us):
  * bf16 inputs AND bf16 output (host casts) -> HBM traffic halved
    (~10.5 MB/core).  PE rate is 1 column/cycle for both fp32r and bf16,
    so bf16 costs no compute -- it only cuts DMA.  Rel-err ~2.9e-3 << 2e-2.
  * WEIGHT-stationary matmuls: lhsT = wT tile [128k x 128 d_out], moving
    rhs = xT [128k x m tokens].  The moving dim is tokens, so the token
    count needs NO 128-row padding: PE work is 16nt x 8kc x m_pad cols
    ~= 57us/core vs 61.4us for the padded x-stationary layout.
  * Weights are HOST-PACKED into the exact SBUF layout ([partition, free]
    with kc-major 128-col runs per nt) so every weight DMA is a fully
    contiguous per-partition run (4KB) -- the naive strided gather ran at
    ~200 GB/s and starved the PE for ~8us mid-kernel.
  * Output is written as yT [d_out, m_pad] (psum already holds yT tiles);
    the host transposes back.  Stores stream out per nt-pair during
    compute on the scalar HWDGE ring; loads use the sync ring.  The last
    nt is stored per-512-block so the final store (and its ~2us HBM
    receipt) is small.
  * One 3-bank PSUM tile + ONE cast per nt (instead of 3) keeps the
    instruction/semaphore count down.
  * PE warm-up: dummy matmuls on a memset scratch tile run during the
    initial DMA fill so the HAM clock-gate is at 8/8 when real work lands.
"""

import math

import numpy as np

_N_CORES = 8
_P = 128
_MBLK = 512  # PSUM bank width in fp32

_program_cache = {}


def _build_program(m_pad, d_in, d_out, warm_mms=8):
    import concourse.mybir as mybir
    import concourse.tile as tile
    from concourse import bacc

    kc_n = d_in // _P           # contraction chunks of 128
    nt_n = d_out // _P          # output-feature tiles of 128
    blocks = [(s, min(s + _MBLK, m_pad)) for s in range(0, m_pad, _MBLK)]
    nblk = len(blocks)

    nc = bacc.Bacc("TRN2", target_bir_lowering=False, debug=False)
    xT = nc.dram_tensor("xT", [d_in, m_pad], mybir.dt.bfloat16, kind="ExternalInput")
    # wP: host-packed weights. wP[p, (nt*kc_n + kc)*128 + j] =
    #     weight[e].T[kc*128 + p, nt*128 + j]
    wP = nc.dram_tensor("wP", [_P, nt_n * kc_n * _P], mybir.dt.bfloat16,
                        kind="ExternalInput")
    yT = nc.dram_tensor("yT", [d_out, m_pad], mybir.dt.bfloat16,
                        kind="ExternalOutput")

    xT3 = xT.rearrange("(kc p) m -> kc p m", p=_P)
    yT3 = yT.rearrange("(nt p) m -> nt p m", p=_P)

    # weight DMA groups (by nt): two singles up front so the first matmuls
    # aren't gated on a big transfer, pairs after.
    wgroups = []
    nt = 0
    while nt < nt_n:
        g = 1 if len(wgroups) < 2 else 2
        g = min(g, nt_n - nt)
        wgroups.append((nt, nt + g))
        nt += g
    gi_of_nt = {}
    for gi, (n0, n1) in enumerate(wgroups):
        for t in range(n0, n1):
            gi_of_nt[t] = gi

    with tile.TileContext(nc) as tc:
        with (
            tc.tile_pool(name="xw", bufs=1) as xwpool,
            tc.tile_pool(name="out", bufs=4) as outpool,
            tc.tile_pool(name="psum", bufs=7, space="PSUM") as psumpool,
            tc.tile_pool(name="wps", bufs=1, space="PSUM") as wpspool,
        ):
            # ---- PE warm-up on a zeroed scratch tile (no data deps) ----
            if warm_mms:
                scratch = xwpool.tile([_P, 640], mybir.dt.bfloat16, tag="scratch")
                nc.gpsimd.memset(scratch[:], 0.0)
                wps = wpspool.tile([_P, _MBLK], mybir.dt.float32, tag="wps")
                for _ in range(warm_mms):
                    nc.tensor.matmul(wps[:], scratch[:, 0:_P], scratch[:, _P:640],
                                     start=True, stop=True)

            # ---- input DMAs (sync/SP HWDGE ring), in PE need-order ----
            xsb = xwpool.tile([_P, kc_n * m_pad], mybir.dt.bfloat16, tag="x")
            wsb = {}

            def load_w(gi):
                n0, n1 = wgroups[gi]
                t = xwpool.tile([_P, (n1 - n0) * kc_n * _P], mybir.dt.bfloat16,
                                tag=f"w{gi}")
                nc.sync.dma_start(t[:], wP[:, n0 * kc_n * _P:n1 * kc_n * _P])
                wsb[gi] = t

            nc.sync.dma_start(xsb[:, 0:m_pad], xT3[0])
            load_w(0)
            for kc in range(1, kc_n):
                nc.sync.dma_start(xsb[:, kc * m_pad:(kc + 1) * m_pad], xT3[kc])
            for gi in range(1, len(wgroups)):
                load_w(gi)

            # ---- compute + cast + store ----
            # stores per nt-pair; the final nt per-block so the last store
            # is small.
            for t in range(nt_n):
                gi = gi_of_nt[t]
                n0 = wgroups[gi][0]
                ps = [psumpool.tile([_P, _MBLK], mybir.dt.float32,
                                    tag="ps", name=f"ps_{t}_{bi}")
                      for bi in range(nblk)]
                for kc in range(kc_n):
                    off = ((t - n0) * kc_n + kc) * _P
                    lhsT = wsb[gi][:, off:off + _P]
                    for bi, (s, e) in enumerate(blocks):
                        nc.tensor.matmul(
                            ps[bi][:, :e - s],
                            lhsT,
                            xsb[:, kc * m_pad + s:kc * m_pad + e],
                            start=(kc == 0),
                            stop=(kc == kc_n - 1),
                        )
                last = (t == nt_n - 1)
                # pair stores only when both halves will be produced before
                # the last nt (which stores per-block)
                paired = (t // 2) * 2 + 1 <= nt_n - 2
                if not last:
                    if not paired:
                        o = outpool.tile([_P, m_pad], mybir.dt.bfloat16,
                                         tag="o", name=f"o_{t}")
                        o_half = 0
                    elif t % 2 == 0:
                        o = outpool.tile([_P, 2 * m_pad], mybir.dt.bfloat16,
                                         tag="o", name=f"o_{t}")
                        o_half = 0
                    else:
                        o_half = 1
                    dst = o[:, o_half * m_pad:(o_half + 1) * m_pad]
                    for bi, (s, e) in enumerate(blocks):
                        nc.vector.tensor_copy(dst[:, s:e], ps[bi][:, :e - s])
                    if paired and t % 2 == 1:
                        nc.scalar.dma_start(
                            yT3[t - 1:t + 1].rearrange("t p m -> p t m"),
                            o[:].rearrange("p (t m) -> p t m", t=2),
                        )
                    elif not paired:
                        nc.scalar.dma_start(yT3[t], o[:])
                else:
                    # last nt: per-block cast+store so the tail store is tiny
                    ol = outpool.tile([_P, m_pad], mybir.dt.bfloat16,
                                      tag="ol", name="o_last")
                    for bi, (s, e) in enumerate(blocks):
                        nc.vector.tensor_copy(
                            ol[:, s:e], ps[bi][:, :e - s])
                        nc.scalar.dma_start(yT3[t][:, s:e], ol[:, s:e])
    nc.compile()
    return nc


# Largest chunk one core handles per SPMD round.
_MAX_CHUNK = 2560


def _plan_chunks(m_sizes, T):
    """Split [0, T) into single-expert chunks, balanced by length.

    Every chunk is <= _MAX_CHUNK rows.  Returns a list of (expert, row0,
    row1) padded with empty (0, 0, 0) chunks to a multiple of _N_CORES,
    or None if there are no rows at all.
    """
    off = np.cumsum(np.asarray(m_sizes, dtype=np.int64))
    starts = np.clip(np.concatenate([[0], off[:-1]]), 0, T)
    ends = np.clip(off, 0, T)
    segs = [(e, int(starts[e]), int(ends[e]))
            for e in range(len(m_sizes)) if ends[e] > starts[e]]
    if not segs:
        return None
    lens = np.array([s1 - s0 for _, s0, s1 in segs], dtype=np.float64)
    n_chunks = np.ceil(lens / _MAX_CHUNK).astype(np.int64)
    total = int(n_chunks.sum())
    spare = (-total) % _N_CORES if total > _N_CORES else _N_CORES - total
    for _ in range(spare):
        i = int(np.argmax(lens / n_chunks))
        n_chunks[i] += 1
    chunks = []
    for (e, s0, s1), k in zip(segs, n_chunks):
        L = s1 - s0
        bounds = [s0 + (L * i) // k for i in range(int(k) + 1)]
        for i in range(int(k)):
            if bounds[i + 1] > bounds[i]:
                chunks.append((e, bounds[i], bounds[i + 1]))
    while len(chunks) % _N_CORES:
        chunks.append((0, 0, 0))
    return chunks


def _pack_weight(wT_e, kc_n, nt_n):
    """wT_e [d_in, d_out] -> [128, nt*kc*128] in the SBUF layout."""
    d_in, d_out = wT_e.shape
    w4 = wT_e.reshape(kc_n, _P, nt_n, _P)          # [kc, p, nt, j]
    return np.ascontiguousarray(
        w4.transpose(1, 2, 0, 3).reshape(_P, nt_n * kc_n * _P))


def kernel(x, weight, m_sizes):
    import ml_dtypes
    from concourse.bass_utils import run_bass_kernel_spmd

    bf16 = ml_dtypes.bfloat16
    x = np.ascontiguousarray(np.asarray(x), dtype=np.float32)
    weight = np.ascontiguousarray(np.asarray(weight), dtype=np.float32)
    m_arr = np.asarray(m_sizes)

    T, d_in = x.shape
    E, d_out, _ = weight.shape

    y = np.zeros((T, d_out), dtype=np.float32)
    chunks = _plan_chunks(m_arr, T)
    if chunks is None:
        return y

    max_len = max(r1 - r0 for _, r0, r1 in chunks)
    m_pad = max(8, int(math.ceil(max_len / 4)) * 4)

    import os
    warm_mms = int(os.environ.get("MOE_WARM_MMS", "8"))
    key = (m_pad, d_in, d_out, warm_mms)
    if key not in _program_cache:
        _program_cache[key] = _build_program(m_pad, d_in, d_out, warm_mms)
    nc = _program_cache[key]

    kc_n = d_in // _P
    nt_n = d_out // _P
    wP_cache = {}
    for round0 in range(0, len(chunks), _N_CORES):
        batch = chunks[round0:round0 + _N_CORES]
        in_maps = []
        for e, r0, r1 in batch:
            xT = np.zeros((d_in, m_pad), dtype=bf16)
            if r1 > r0:
                xT[:, : r1 - r0] = x[r0:r1].T
            if e not in wP_cache:
                wP_cache[e] = _pack_weight(
                    np.ascontiguousarray(weight[e].T).astype(bf16), kc_n, nt_n)
            in_maps.append({"xT": xT, "wP": wP_cache[e]})

        res = run_bass_kernel_spmd(nc, in_maps, core_ids=list(range(_N_CORES)))

        for (e, r0, r1), out in zip(batch, res.results):
            if r1 > r0:
                y[r0:r1] = out["yT"][:, : r1 - r0].T.astype(np.float32)
    return y
